# revision 1
# baseline (speedup 1.0000x reference)
"""Multi-head attention (B=1, S=4096, D=768, H=12, Hd=64) on 8 trn2 cores.

Sharding: 2 head-groups (6 heads = 384 dims, Megatron column-split wq/wk/wv,
row-split ww) x 4 query-chunks (1024 rows).  core = g*4 + c.
Each core returns a partial output [1024, 768]; host sums the 2 group
partials per chunk and adds (bv @ ww.T + bw).

Per-core plan:
  xT  [768, 4096]   x transposed (keys/values source), xqT = query columns.
  QT/KT [128, 3, *] head-pair-packed transposed projections: partition
                    l, pair p -> local dim p*128+l.  The two heads of a pair
                    run their scores matmuls concurrently in the PE array via
                    contraction row-packing (base partitions 0 / 64).
  V2  [128, 32, 6, 65]  value rows (key j on partitions) per head, with a
                    ones column at index 64: the attnV matmul (M=65) then
                    accumulates both out^T (rows 0-63) and the softmax
                    denominator (row 64) over key tiles in PSUM.
  scoresT psum [128 keys, 512 q]; exp on ACT engine psum->SBUF (x1/8 folded
                    into the activation scale; no max subtraction needed:
                    |scores| < 3).
  y6  [64, 6, 1024] normalized attn output^T per head (64 partitions), so
                    no partition shifts are needed; out-proj contracts 6x64.
All matmul inputs are float32r (full-rate fp32 mode, moving dim >= 256).
"""

import sys

if "/opt/trn_rl_repo" not in sys.path:
    sys.path.insert(0, "/opt/trn_rl_repo")

import numpy as np

import concourse.bacc as bacc
import concourse.bass as bass
import concourse.mybir as mybir
import concourse.tile as tile
from concourse.bass_utils import run_bass_kernel_spmd
from concourse.vector_clock import ScopedClock

F32 = mybir.dt.float32
F32R = mybir.dt.float32r
BF16 = mybir.dt.bfloat16
import os
MD = {"f32r": F32R, "bf16": BF16, "f32": F32}[os.environ.get("MM_DTYPE", "f32r")]
MERGED_EXP = os.environ.get("MERGED_EXP", "1") == "1"

S = 4096          # sequence length
D = 768           # model dim
NG = 2            # head groups (cores axis 1)
NC = 4            # query chunks (cores axis 2)
DH = D // NG      # dims per group = 384
NP = DH // 128    # head pairs per group = 3
NH = 2 * NP       # heads per group = 6
SQ = S // NC      # queries per core = 1024
KO = D // 128     # contraction subtiles = 6
NJ = S // 128     # key tiles = 32
AF = mybir.ActivationFunctionType
SCALE = 0.125     # 1/sqrt(64)
CHUNKS = [3] * 10 + [2]   # 32 key tiles in exp-sized chunks

_PATCHED = False


def _patch_drain():
    """walrus in this container rejects >1 sync-wait per instruction
    ("Too many sync wait commands").  TileContext's tail drain aggregates one
    wait per live tile semaphore; redistribute them one-per-nop.  (Bacc's
    generate_event_semaphores handles the rest of the kernel.)"""
    global _PATCHED
    if _PATCHED:
        return
    _PATCHED = True

    def _drain_and_barrier(self, tick_clock, wait_clock):
        nc = self.nc
        drain_inst = nc.sync.drain()
        wait_clock.add_sem_waits(
            drain_inst.ins, ScopedClock({None: tick_clock.global_clock})
        )
        si = drain_inst.ins.sync_info
        waits = list(si.on_wait) if si is not None else []
        if len(waits) > 1:
            drain_inst.ins.sync_info = mybir.SyncInfo(
                on_wait=[waits[0]], on_update=list(si.on_update)
            )
            for w in waits[1:]:
                nop = nc.sync.nop(nofuse=True)
                nop.ins.sync_info = mybir.SyncInfo(on_wait=[w], on_update=[])
        nc.all_engine_barrier()
        assert self.sems is not None
        popped = nc._tile_sem_poison_stack.pop()
        assert popped is self._sem_poison
        nc.clear_and_free_semaphores(list(self.sems.allocated().values()))
        nc.all_engine_barrier()

    tile.TileContext._drain_and_barrier = _drain_and_barrier


def build_nc(loop_n=None):
    _patch_drain()
    nc = bacc.Bacc("TRN2", target_bir_lowering=False)

    xT = nc.dram_tensor("xT", [D, S], MD, kind="ExternalInput")
    xqT = nc.dram_tensor("xqT", [D, SQ], MD, kind="ExternalInput")
    wqT = nc.dram_tensor("wqT", [D, DH], MD, kind="ExternalInput")
    wkT = nc.dram_tensor("wkT", [D, DH], MD, kind="ExternalInput")
    wvT = nc.dram_tensor("wvT", [D, DH], MD, kind="ExternalInput")
    wwT = nc.dram_tensor("wwT", [DH, D], MD, kind="ExternalInput")
    bq = nc.dram_tensor("bq", [128, NP], F32, kind="ExternalInput")
    bk = nc.dram_tensor("bk", [128, NP], F32, kind="ExternalInput")
    out = nc.dram_tensor("out", [SQ, D], F32, kind="ExternalOutput")

    xT_r = xT.rearrange("(ko p) n -> p ko n", p=128)
    xqT_r = xqT.rearrange("(ko p) n -> p ko n", p=128)
    wqT_r = wqT.rearrange("(ko p) m -> p ko m", p=128)
    wkT_r = wkT.rearrange("(ko p) m -> p ko m", p=128)
    wvT_r = wvT.rearrange("(ko p) m -> p ko m", p=128)
    ww6_r = wwT.rearrange("(h l) o -> l h o", l=64)   # [64, 6, 768]

    with tile.TileContext(nc) as tc:
        import contextlib

        with contextlib.ExitStack() as ctx:
            if loop_n is not None:
                ctx.enter_context(tc.For_i(0, loop_n, 1))
            persist = ctx.enter_context(tc.tile_pool(name="persist", bufs=1))
            KT = persist.tile([128, NP, S], MD)        # 48KB/part
            # flat per-head V: cols j*65..j*65+63 = V rows, col j*65+64 = 1.0;
            # 63-col tail so the M=128 attnV lhsT AP may overrun harmlessly.
            V2 = persist.tile([128, NH, NJ * 65 + 63], MD)   # 51.4KB/part
            # per-head zero-padded Q^T: full-K(128) scores matmuls read the
            # whole head pair as lhsT; zeros in the complementary half kill
            # the cross-head term.  (K<128 matmuls run at half rate.)
            QTz = persist.tile([128, NH, SQ], MD)      # 24KB/part
            ones_f32 = persist.tile([128, 1], F32)
            zero_f32 = persist.tile([128, 1], F32)
            nc.vector.memset(ones_f32[:], 1.0)
            nc.vector.memset(zero_f32[:], 0.0)
            for h in range(NH):
                v2h = V2[:, h, 0:NJ * 65].rearrange("l (j c) -> l j c", c=65)
                nc.vector.tensor_copy(
                    v2h[:, :, 64:65],
                    ones_f32[:, 0:1].to_broadcast((128, NJ, 1)),
                )
                nc.vector.tensor_copy(
                    V2[:, h, NJ * 65:],
                    zero_f32[:, 0:1].to_broadcast((128, 63)),
                )
                # zero the complementary contraction half of QTz
                if h % 2 == 0:
                    nc.vector.tensor_copy(
                        QTz[64:128, h, :],
                        zero_f32[64:128, 0:1].to_broadcast((64, SQ)),
                    )
                else:
                    nc.vector.tensor_copy(
                        QTz[0:64, h, :],
                        zero_f32[0:64, 0:1].to_broadcast((64, SQ)),
                    )

            with tc.tile_pool(name="proj", bufs=1) as proj, \
                 tc.tile_pool(name="ps12", bufs=3, space="PSUM") as ps12, \
                 tc.tile_pool(name="psq", bufs=2, space="PSUM") as psq:
                wk_sb = proj.tile([128, KO, DH], MD)
                wv_sb = proj.tile([128, KO, DH], MD)
                wq_sb = proj.tile([128, KO, DH], MD)
                xq_sb = proj.tile([128, KO, SQ], MD)
                bq_sb = proj.tile([128, NP], F32)
                bk_sb = proj.tile([128, NP], F32)
                nc.sync.dma_start(wk_sb[:], wkT_r[:])
                nc.sync.dma_start(bk_sb[:], bk[:])
                nc.sync.dma_start(wv_sb[:], wvT_r[:])

                # ------------- phase 1: K/V projections (stream xT) ------
                with tc.tile_pool(name="xstream", bufs=2) as xs_pool:
                    for n in range(S // 512):
                        xb = xs_pool.tile([128, KO, 512], MD, tag="xb")
                        nc.sync.dma_start(xb[:], xT_r[:, :, n * 512:(n + 1) * 512])
                        for p in range(NP):
                            ps = ps12.tile([128, 512], F32, tag="qk")
                            for ko in range(KO):
                                nc.tensor.matmul(
                                    ps[:],
                                    wk_sb[:, ko, p * 128:(p + 1) * 128],
                                    xb[:, ko, :],
                                    start=(ko == 0), stop=(ko == KO - 1),
                                )
                            nc.vector.tensor_scalar_add(
                                KT[:, p, n * 512:(n + 1) * 512], ps[:],
                                bk_sb[:, p:p + 1],
                            )
                        for j4 in range(4):
                            j = n * 4 + j4
                            ps = ps12.tile([128, 512], F32, tag="v")
                            for ko in range(KO):
                                nc.tensor.matmul(
                                    ps[:, :DH],
                                    xb[:, ko, j4 * 128:(j4 + 1) * 128],
                                    wv_sb[:, ko, :],
                                    start=(ko == 0), stop=(ko == KO - 1),
                                )
                            for h in range(NH):
                                nc.vector.tensor_copy(
                                    V2[:, h, j * 65:j * 65 + 64],
                                    ps[:, h * 64:(h + 1) * 64],
                                )
                        if n == 0:
                            # deferred so they don't delay the first x block
                            nc.sync.dma_start(wq_sb[:], wqT_r[:])
                            nc.sync.dma_start(xq_sb[:], xqT_r[:])
                            nc.sync.dma_start(bq_sb[:], bq[:])
                        if n == 2:
                            # Q projection emitted mid-stream: its own psum
                            # tag lets it fill PE gaps during xT DMA waits
                            for p in range(NP):
                                for nq in range(SQ // 512):
                                    nqs = slice(nq * 512, (nq + 1) * 512)
                                    psq_t = psq.tile([128, 512], F32, tag="q")
                                    for ko in range(KO):
                                        nc.tensor.matmul(
                                            psq_t[:],
                                            wq_sb[:, ko, p * 128:(p + 1) * 128],
                                            xq_sb[:, ko, nqs],
                                            start=(ko == 0), stop=(ko == KO - 1),
                                        )
                                    nc.vector.tensor_scalar_add(
                                        QTz[0:64, 2 * p, nqs], psq_t[0:64, :],
                                        bq_sb[0:64, p:p + 1],
                                    )
                                    nc.vector.tensor_scalar_add(
                                        QTz[64:128, 2 * p + 1, nqs], psq_t[64:128, :],
                                        bq_sb[64:128, p:p + 1],
                                    )

            # ---------------- phases 3+4 ----------------
            with tc.tile_pool(name="late", bufs=1) as late, \
                 tc.tile_pool(name="pt", bufs=2) as pt_pool, \
                 tc.tile_pool(name="dn", bufs=2) as dn_pool, \
                 tc.tile_pool(name="bc", bufs=2) as bc_pool, \
                 tc.tile_pool(name="ob", bufs=2) as ob_pool, \
                 tc.tile_pool(name="ps_sc", bufs=1, space="PSUM") as ps_sc, \
                 tc.tile_pool(name="ps_out", bufs=1, space="PSUM") as ps_out:
                # [128, ...] with zeroed rows 64-127: full-K out-proj.
                # ([64, x] tiles reserve the same per-partition bytes anyway.)
                y6 = late.tile([128, NH, SQ], MD)      # 24KB/part
                ww6 = late.tile([128, NH, D], MD)      # 18KB/part
                nc.sync.dma_start(ww6[0:64, :, :], ww6_r[:])
                nc.vector.tensor_copy(
                    y6[64:128, :, :].rearrange("l h q -> l (h q)"),
                    zero_f32[64:128, 0:1].to_broadcast((64, NH * SQ)),
                )
                nc.vector.tensor_copy(
                    ww6[64:128, :, :].rearrange("l h o -> l (h o)"),
                    zero_f32[64:128, 0:1].to_broadcast((64, NH * D)),
                )

                for qh in range(SQ // 512):
                    for p in range(NP):
                        qs = slice(qh * 512, (qh + 1) * 512)
                        oA = ps_out.tile([128, 512], F32, tag="outA")
                        oB = ps_out.tile([128, 512], F32, tag="outB")
                        j0 = 0
                        for cs in CHUNKS:
                            scA = ps_sc.tile([128, 3, 512], F32, tag="scA")
                            scB = ps_sc.tile([128, 3, 512], F32, tag="scB")
                            for t in range(cs):
                                j = j0 + t
                                js = slice(j * 128, (j + 1) * 128)
                                nc.tensor.matmul(
                                    scA[:, t, :],
                                    KT[:, p, js], QTz[:, 2 * p, qs],
                                    start=True, stop=True,
                                )
                            for t in range(cs):
                                j = j0 + t
                                js = slice(j * 128, (j + 1) * 128)
                                nc.tensor.matmul(
                                    scB[:, t, :],
                                    KT[:, p, js], QTz[:, 2 * p + 1, qs],
                                    start=True, stop=True,
                                )
                            ptA = pt_pool.tile([128, 3, 512], MD, tag="ptA")
                            ptB = pt_pool.tile([128, 3, 512], MD, tag="ptB")
                            nc.scalar.activation(
                                ptA[:, :cs, :], scA[:, :cs, :], AF.Exp, scale=SCALE
                            )
                            nc.scalar.activation(
                                ptB[:, :cs, :], scB[:, :cs, :], AF.Exp, scale=SCALE
                            )
                            for t in range(cs):
                                j = j0 + t
                                nc.tensor.matmul(
                                    oA[:, :],
                                    V2[:, 2 * p, j * 65:j * 65 + 128],
                                    ptA[:, t, :],
                                    start=(j == 0), stop=(j == NJ - 1),
                                )
                            for t in range(cs):
                                j = j0 + t
                                nc.tensor.matmul(
                                    oB[:, :],
                                    V2[:, 2 * p + 1, j * 65:j * 65 + 128],
                                    ptB[:, t, :],
                                    start=(j == 0), stop=(j == NJ - 1),
                                )
                            j0 += cs
                        # normalize: row 64 holds the softmax denominator
                        for h, o_ps in ((2 * p, oA), (2 * p + 1, oB)):
                            dn = dn_pool.tile([1, 512], F32, tag="dn")
                            nc.vector.tensor_copy(dn[:], o_ps[64:65, :])
                            bc = bc_pool.tile([64, 512], F32, tag="bc")
                            nc.gpsimd.partition_broadcast(bc[:], dn[:], channels=64)
                            nc.vector.reciprocal(bc[:], bc[:])
                            nc.vector.tensor_mul(
                                y6[0:64, h, qs], o_ps[0:64, :], bc[:]
                            )

                    # ---------- phase 4: out-projection for this q-half ----
                    for m in range(qh * 4, (qh + 1) * 4):
                        ms = slice(m * 128, (m + 1) * 128)
                        ob = ob_pool.tile([128, D], F32, tag="ob")
                        for n0, nw in ((0, 512), (512, 256)):
                            ps = ps_out.tile([128, 512], F32, tag="outA")
                            for h in range(NH):
                                nc.tensor.matmul(
                                    ps[:, :nw],
                                    y6[:, h, ms],
                                    ww6[:, h, n0:n0 + nw],
                                    start=(h == 0), stop=(h == NH - 1),
                                )
                            nc.vector.tensor_copy(ob[:, n0:n0 + nw], ps[:, :nw])
                        nc.sync.dma_start(out[ms, :], ob[:])

    nc.finalize()  # Bacc.compile(): reg alloc + split multi-sem-waits
    return nc


_NC_CACHE = None


def make_in_maps(x, wq, bq, wk, bk, wv, ww):
    npdt = mybir.dt.np(MD)
    x = np.ascontiguousarray(np.asarray(x, dtype=np.float32))
    xT_full = np.ascontiguousarray(x[0].T).astype(npdt)  # [D, S]
    in_maps = []
    for core in range(8):
        g, c = core // NC, core % NC
        gs = slice(g * DH, (g + 1) * DH)
        in_maps.append({
            "xT": xT_full,
            "xqT": np.ascontiguousarray(xT_full[:, c * SQ:(c + 1) * SQ]),
            "wqT": np.ascontiguousarray(wq[gs, :].T).astype(npdt),
            "wkT": np.ascontiguousarray(wk[gs, :].T).astype(npdt),
            "wvT": np.ascontiguousarray(wv[gs, :].T).astype(npdt),
            "wwT": np.ascontiguousarray(ww[:, gs].T).astype(npdt),
            "bq": np.ascontiguousarray(bq[gs].reshape(NP, 128).T),
            "bk": np.ascontiguousarray(bk[gs].reshape(NP, 128).T),
        })
    return in_maps


def kernel(x, wq, bq, wk, bk, wv, bv, ww, bw):
    global _NC_CACHE
    if _NC_CACHE is None:
        _NC_CACHE = build_nc()
    nc = _NC_CACHE

    in_maps = make_in_maps(x, wq, bq, wk, bk, wv, ww)
    res = run_bass_kernel_spmd(nc, in_maps, core_ids=list(range(8)))

    const_row = (bv @ ww.T + bw).astype(np.float32)  # [768]
    out = np.empty((1, S, D), dtype=np.float32)
    for c in range(NC):
        acc = res.results[0 * NC + c]["out"] + res.results[1 * NC + c]["out"]
        out[0, c * SQ:(c + 1) * SQ, :] = acc + const_row
    return out



# revision 8
# speedup vs baseline: 1.2957x; 1.2957x over previous
"""Multi-head attention (B=1, S=4096, D=768, H=12, Hd=64) on 8 trn2 cores.

Sharding: 2 head-groups (6 heads = 384 dims, Megatron column-split wq/wk/wv,
row-split ww) x 4 query-chunks (1024 rows).  core = g*4 + c.
Each core returns a partial output [1024, 768]; host sums the 2 group
partials per chunk and adds (bv @ ww.T + bw).

v2 design (ACT-bound fused schedule):
  - All projections / scores in bf16 (1 cyc/row on PE, exact-enough).
  - attnV in fp8e4 DoubleRow (2 key-tiles per instruction): V8 holds
    fp8(64*V) rows per key with a ones column at 64 and 63 pad cols (dual-fp8
    ldweights requires the full 128 weight columns); pt = fp8(8*exp(s/8))
    written directly by the ACT exp.  Scale bookkeeping: numerator rows are
    512*(P.V), denominator row 64 is 8*sum(P) -> y6 = 64*out; ww is
    pre-divided by 64 on the host.
  - The exp stream on ACT (164us of columns) is the binding engine.  The
    key axis is split in NSPLIT=4 quarters; within each quarter the 12
    rounds (qh, p, head) run scores->exp->attnV pipelined through a
    double-buffered [128,3,512] psum pair, while K/V projection blocks
    (quarters 0-2) and the out-projection (quarter 3) execute in the PE
    gaps as interleaved "filler" pieces.  attnV accumulates per-quarter in
    a single psum bank and spills/accumulates into acc (SBUF, f32).
  - psum budget: scores 2x3 banks + o 1 + filler 1 = 8.
"""

import sys

if "/opt/trn_rl_repo" not in sys.path:
    sys.path.insert(0, "/opt/trn_rl_repo")

import math
from collections import deque

import numpy as np
import ml_dtypes

import concourse.bacc as bacc
import concourse.bass as bass
import concourse.mybir as mybir
import concourse.tile as tile
from concourse.bass_utils import run_bass_kernel_spmd
from concourse.vector_clock import ScopedClock

F32 = mybir.dt.float32
BF = mybir.dt.bfloat16
F8 = mybir.dt.float8e4
AF = mybir.ActivationFunctionType
DR = mybir.MatmulPerfMode.DoubleRow

S = 4096          # sequence length
D = 768           # model dim
NG = 2            # head groups (cores axis 1)
NC = 4            # query chunks (cores axis 2)
DH = D // NG      # dims per group = 384
NP = DH // 128    # head pairs per group = 3
NH = 2 * NP       # heads per group = 6
SQ = S // NC      # queries per core = 1024
KO = D // 128     # contraction subtiles = 6
NJ = S // 128     # key tiles = 32
SCALE = 0.125     # 1/sqrt(64)
LN8 = float(math.log(8.0))
VSCALE = 32.0     # host scale folded into wv (and 1/VSCALE into ww);
                  # max |VSCALE*v| ~ 127 stays below the TRN e4m3 max of 240
                  # (the DVE f32->fp8 conversion overflows instead of
                  # saturating, so headroom is required)

NSPLIT = 4        # key-axis quarters
JQ = NJ // NSPLIT           # j-tiles per quarter = 8
NPAIR = JQ // 2             # DoubleRow pairs per round-quarter = 4
QCHUNKS = [3, 3, 2]         # exp chunk sizes covering JQ j-tiles
ROUNDS = [(qh, p, h) for qh in range(2) for p in range(NP) for h in range(2)]

_PATCHED = False


def _patch_drain():
    """walrus in this container rejects >1 sync-wait per instruction
    ("Too many sync wait commands").  TileContext's tail drain aggregates one
    wait per live tile semaphore; redistribute them one-per-nop.  (Bacc's
    generate_event_semaphores handles the rest of the kernel.)"""
    global _PATCHED
    if _PATCHED:
        return
    _PATCHED = True

    def _drain_and_barrier(self, tick_clock, wait_clock):
        nc = self.nc
        drain_inst = nc.sync.drain()
        wait_clock.add_sem_waits(
            drain_inst.ins, ScopedClock({None: tick_clock.global_clock})
        )
        si = drain_inst.ins.sync_info
        waits = list(si.on_wait) if si is not None else []
        if len(waits) > 1:
            drain_inst.ins.sync_info = mybir.SyncInfo(
                on_wait=[waits[0]], on_update=list(si.on_update)
            )
            for w in waits[1:]:
                nop = nc.sync.nop(nofuse=True)
                nop.ins.sync_info = mybir.SyncInfo(on_wait=[w], on_update=[])
        nc.all_engine_barrier()
        assert self.sems is not None
        popped = nc._tile_sem_poison_stack.pop()
        assert popped is self._sem_poison
        nc.clear_and_free_semaphores(list(self.sems.allocated().values()))
        nc.all_engine_barrier()

    tile.TileContext._drain_and_barrier = _drain_and_barrier


def build_nc(loop_n=None, debug=False):
    _patch_drain()
    nc = bacc.Bacc("TRN2", target_bir_lowering=False)

    xT = nc.dram_tensor("xT", [D, S], BF, kind="ExternalInput")
    xqT = nc.dram_tensor("xqT", [D, SQ], BF, kind="ExternalInput")
    wqT = nc.dram_tensor("wqT", [D, DH], BF, kind="ExternalInput")
    wkT = nc.dram_tensor("wkT", [D, DH], BF, kind="ExternalInput")
    wvT = nc.dram_tensor("wvT", [D, DH], BF, kind="ExternalInput")  # x VSCALE
    wwT = nc.dram_tensor("wwT", [DH, D], BF, kind="ExternalInput")  # / VSCALE
    bq = nc.dram_tensor("bq", [128, NP], F32, kind="ExternalInput")
    bk = nc.dram_tensor("bk", [128, NP], F32, kind="ExternalInput")
    out = nc.dram_tensor("out", [SQ, D], F32, kind="ExternalOutput")

    xT_r = xT.rearrange("(ko p) n -> p ko n", p=128)
    xqT_r = xqT.rearrange("(ko p) n -> p ko n", p=128)
    wqT_r = wqT.rearrange("(ko p) m -> p ko m", p=128)
    wkT_r = wkT.rearrange("(ko p) m -> p ko m", p=128)
    wvT_r = wvT.rearrange("(ko p) m -> p ko m", p=128)
    ww6_r = wwT.rearrange("(h l) o -> l h o", l=64)   # [64, 6, 768]

    with tile.TileContext(nc) as tc:
        import contextlib

        with contextlib.ExitStack() as ctx:
            if loop_n is not None:
                ctx.enter_context(tc.For_i(0, loop_n, 1))
            persist = ctx.enter_context(tc.tile_pool(name="persist", bufs=1))
            KT = persist.tile([128, NP, S], BF)         # 24KB/part
            QTz = persist.tile([128, NH, SQ], BF)       # 12KB/part
            V8 = persist.tile([128, NH, NJ, 128], F8)   # 24KB/part
            acc = persist.tile([128, 12, 512], F32)     # 24KB/part
            ptr = persist.tile([128, 2, 6, 512], F8)    # exp rings, 6KB/part
            y6 = persist.tile([128, NH, SQ], BF)        # 12KB/part
            ww6 = persist.tile([128, NH, D], BF)        # 9KB/part
            lnb = persist.tile([128, 1], F32)
            bq_sb = persist.tile([128, NP], F32)
            bk_sb = persist.tile([128, NP], F32)
            zero_bf = persist.tile([128, 1], BF)

            w_pool = ctx.enter_context(tc.tile_pool(name="w", bufs=1))
            wk_sb = w_pool.tile([128, KO, DH], BF)
            wv_sb = w_pool.tile([128, KO, DH], BF)
            wq_sb = w_pool.tile([128, KO, DH], BF)

            xs = ctx.enter_context(tc.tile_pool(name="xs", bufs=3))
            ob_pool = ctx.enter_context(tc.tile_pool(name="ob", bufs=2))
            bc_pool = ctx.enter_context(tc.tile_pool(name="bc", bufs=2))

            sc_pool = ctx.enter_context(
                tc.tile_pool(name="sc", bufs=2, space="PSUM"))
            o_pool = ctx.enter_context(
                tc.tile_pool(name="o", bufs=1, space="PSUM"))
            kv_pool = ctx.enter_context(
                tc.tile_pool(name="kv", bufs=1, space="PSUM"))

            # ---------------- init + weight DMA ----------------
            nc.sync.dma_start(wk_sb[:], wkT_r[:])
            nc.sync.dma_start(bk_sb[:], bk[:])
            nc.sync.dma_start(wv_sb[:], wvT_r[:])
            nc.sync.dma_start(wq_sb[:], wqT_r[:])
            nc.sync.dma_start(bq_sb[:], bq[:])
            nc.sync.dma_start(ww6[0:64, :, :], ww6_r[:])
            nc.vector.memset(lnb[:], LN8)
            nc.vector.memset(zero_bf[:], 0.0)
            # zero-pad halves of QTz kill the cross-head term of the
            # pair-packed K=128 scores matmuls
            for h in range(NH):
                if h % 2 == 0:
                    nc.vector.memset(QTz[64:128, h, :], 0.0)
                else:
                    nc.vector.memset(QTz[0:64, h, :], 0.0)
            # upper halves zero: out-proj runs K=128 over head pairs' rows
            nc.vector.tensor_copy(
                y6[64:128, :, :].rearrange("l h q -> l (h q)"),
                zero_bf[64:128, 0:1].to_broadcast((64, NH * SQ)),
            )
            nc.vector.tensor_copy(
                ww6[64:128, :, :].rearrange("l h o -> l (h o)"),
                zero_bf[64:128, 0:1].to_broadcast((64, NH * D)),
            )

            # ---------------- filler pieces ----------------
            xb_tiles = {}

            def piece_dma_block(n):
                def go():
                    xb = xs.tile([128, KO, 512], BF, tag="xb")
                    xb_tiles[n] = xb
                    nc.sync.dma_start(xb[:], xT_r[:, :, n * 512:(n + 1) * 512])
                    # fp8 pad cols (65:128) + ones col (64) for this block's
                    # j-tiles; gpsimd = off the critical engines
                    nc.gpsimd.memset(V8[:, :, 4 * n:4 * n + 4, 64:128], 0.0)
                    nc.gpsimd.memset(V8[:, :, 4 * n:4 * n + 4, 64:65], 1.0)
                return go

            def piece_k(n, p):
                def go():
                    xb = xb_tiles[n]
                    ps = kv_pool.tile([128, 512], F32, tag="kv")
                    for ko in range(KO):
                        nc.tensor.matmul(
                            ps[:], wk_sb[:, ko, p * 128:(p + 1) * 128],
                            xb[:, ko, :],
                            start=(ko == 0), stop=(ko == KO - 1),
                        )
                    nc.vector.tensor_scalar_add(
                        KT[:, p, n * 512:(n + 1) * 512], ps[:],
                        bk_sb[:, p:p + 1],
                    )
                return go

            def piece_v(n, j4):
                def go():
                    xb = xb_tiles[n]
                    ps = kv_pool.tile([128, 512], F32, tag="kv")
                    for ko in range(KO):
                        nc.tensor.matmul(
                            ps[:, :DH],
                            xb[:, ko, j4 * 128:(j4 + 1) * 128],
                            wv_sb[:, ko, :],
                            start=(ko == 0), stop=(ko == KO - 1),
                        )
                    nc.vector.tensor_copy(
                        V8[:, :, 4 * n + j4, 0:64],
                        ps[:, 0:DH].rearrange("l (h c) -> l h c", c=64),
                    )
                return go

            def block_pieces(n):
                ps = [piece_dma_block(n)]
                for p in range(NP):
                    ps.append(piece_k(n, p))
                for j4 in range(4):
                    ps.append(piece_v(n, j4))
                return ps

            ob_tiles = {}

            def piece_outproj(qh, m, n0, nw):
                def go():
                    ms = slice(m * 128, (m + 1) * 128)
                    ps = kv_pool.tile([128, 512], F32, tag="kv")
                    for h in range(NH):
                        nc.tensor.matmul(
                            ps[:, :nw],
                            y6[:, h, ms],
                            ww6[:, h, n0:n0 + nw],
                            start=(h == 0), stop=(h == NH - 1),
                        )
                    if n0 == 0:
                        ob_tiles[m] = ob_pool.tile(
                            [128, D], F32, tag="ob", name=f"ob{m}")
                    ob = ob_tiles[m]
                    nc.vector.tensor_copy(ob[:, n0:n0 + nw], ps[:, :nw])
                    if n0 + nw == D:
                        nc.sync.dma_start(out[ms, :], ob[:])
                return go

            # ---------------- lead-in ----------------
            for piece in block_pieces(0):
                piece()
            # Q projection (streams xq through the xs pool)
            for nq in range(2):
                if nq == 0:
                    for piece in block_pieces(1):
                        piece()
                xqb = xs.tile([128, KO, 512], BF, tag="xb")
                nc.sync.dma_start(xqb[:], xqT_r[:, :, nq * 512:(nq + 1) * 512])
                nqs = slice(nq * 512, (nq + 1) * 512)
                for p in range(NP):
                    psq = kv_pool.tile([128, 512], F32, tag="kv")
                    for ko in range(KO):
                        nc.tensor.matmul(
                            psq[:], wq_sb[:, ko, p * 128:(p + 1) * 128],
                            xqb[:, ko, :],
                            start=(ko == 0), stop=(ko == KO - 1),
                        )
                    nc.vector.tensor_scalar_add(
                        QTz[0:64, 2 * p, nqs], psq[0:64, :],
                        bq_sb[0:64, p:p + 1],
                    )
                    nc.vector.tensor_scalar_add(
                        QTz[64:128, 2 * p + 1, nqs], psq[64:128, :],
                        bq_sb[64:128, p:p + 1],
                    )

            # ---------------- fused attention quarters ----------------
            for q in range(NSPLIT):
                if q < NSPLIT - 1:
                    fillers = deque(
                        block_pieces(2 * q + 2) + block_pieces(2 * q + 3))
                else:
                    fillers = deque(
                        piece_outproj(0, m, n0, nw)
                        for m in range(4) for (n0, nw) in ((0, 512), (512, 256))
                    )
                # chunk slots where fillers may run (quarter 3: only after
                # the qh=0 rounds are normalized)
                first_fill_round = 6 if q == NSPLIT - 1 else 0
                n_slots = (12 - first_fill_round) * len(QCHUNKS)
                fill_acc = 0.0
                rate = len(fillers) / n_slots

                for r, (qh, p, h) in enumerate(ROUNDS):
                    h6 = 2 * p + h
                    qs = slice(qh * 512, (qh + 1) * 512)
                    ring = ptr[:, r % 2, :, :]
                    o_ps = o_pool.tile([128, 512], F32, tag="o")
                    pair_done = 0
                    jc = 0
                    for c, cs in enumerate(QCHUNKS):
                        sc = sc_pool.tile([128, 3, 512], F32, tag="sc")
                        for t in range(cs):
                            j = q * JQ + jc + t
                            nc.tensor.matmul(
                                sc[:, t, :],
                                KT[:, p, j * 128:(j + 1) * 128],
                                QTz[:, h6, qs],
                                start=True, stop=True,
                            )
                        slot = jc % 6
                        nc.scalar.activation(
                            ring[:, slot:slot + cs, :], sc[:, :cs, :],
                            AF.Exp, scale=SCALE, bias=lnb[:],
                        )
                        jc += cs
                        # attnV for pairs fully exp'd so far
                        while 2 * (pair_done + 1) <= jc:
                            t_l = pair_done
                            jg = q * JQ + 2 * t_l
                            nc.tensor.matmul(
                                o_ps[:],
                                V8[:, h6, jg:jg + 2, :],
                                ring[:, (2 * t_l) % 6:(2 * t_l) % 6 + 2, :],
                                start=(t_l == 0), stop=(t_l == NPAIR - 1),
                                perf_mode=DR,
                            )
                            pair_done += 1
                        if r >= first_fill_round:
                            fill_acc += rate
                            while fill_acc >= 1.0 and fillers:
                                fillers.popleft()()
                                fill_acc -= 1.0
                    # ---- spill / accumulate / normalize ----
                    if q == 0:
                        nc.vector.tensor_copy(acc[0:65, r, :], o_ps[0:65, :])
                    else:
                        nc.vector.tensor_add(
                            acc[0:65, r, :], o_ps[0:65, :], acc[0:65, r, :])
                    if q == NSPLIT - 1:
                        dn = bc_pool.tile([1, 512], F32, tag="dn")
                        nc.vector.tensor_copy(dn[:], acc[64:65, r, :])
                        bc = bc_pool.tile([64, 512], F32, tag="bc")
                        nc.gpsimd.partition_broadcast(bc[:], dn[:], channels=64)
                        nc.vector.reciprocal(bc[:], bc[:])
                        nc.vector.tensor_mul(
                            y6[0:64, h6, qs], acc[0:64, r, :], bc[:])
                while fillers:
                    fillers.popleft()()

            # ---------------- tail: out-proj for qh=1 ----------------
            for m in range(4, 8):
                for (n0, nw) in ((0, 512), (512, 256)):
                    piece_outproj(1, m, n0, nw)()

            if debug:
                dKT = nc.dram_tensor("dKT", [128, NP, S], BF, kind="ExternalOutput")
                dQT = nc.dram_tensor("dQT", [128, NH, SQ], BF, kind="ExternalOutput")
                dV8 = nc.dram_tensor("dV8", [128, NH, NJ, 128], F8, kind="ExternalOutput")
                dacc = nc.dram_tensor("dacc", [128, 12, 512], F32, kind="ExternalOutput")
                dy6 = nc.dram_tensor("dy6", [128, NH, SQ], BF, kind="ExternalOutput")
                nc.sync.dma_start(dKT[:], KT[:])
                nc.sync.dma_start(dQT[:], QTz[:])
                nc.sync.dma_start(dV8[:], V8[:])
                nc.sync.dma_start(dacc[:], acc[:])
                nc.sync.dma_start(dy6[:], y6[:])

    nc.finalize()
    return nc


_NC_CACHE = None


def make_in_maps(x, wq, bq, wk, bk, wv, ww):
    x = np.ascontiguousarray(np.asarray(x, dtype=np.float32))
    xT_full = np.ascontiguousarray(x[0].T).astype(ml_dtypes.bfloat16)  # [D, S]
    in_maps = []
    for core in range(8):
        g, c = core // NC, core % NC
        gs = slice(g * DH, (g + 1) * DH)
        in_maps.append({
            "xT": xT_full,
            "xqT": np.ascontiguousarray(xT_full[:, c * SQ:(c + 1) * SQ]),
            "wqT": np.ascontiguousarray(wq[gs, :].T).astype(ml_dtypes.bfloat16),
            "wkT": np.ascontiguousarray(wk[gs, :].T).astype(ml_dtypes.bfloat16),
            "wvT": np.ascontiguousarray(wv[gs, :].T * VSCALE).astype(ml_dtypes.bfloat16),
            "wwT": np.ascontiguousarray(ww[:, gs].T / VSCALE).astype(ml_dtypes.bfloat16),
            "bq": np.ascontiguousarray(bq[gs].reshape(NP, 128).T).astype(np.float32),
            "bk": np.ascontiguousarray(bk[gs].reshape(NP, 128).T).astype(np.float32),
        })
    return in_maps


def kernel(x, wq, bq, wk, bk, wv, bv, ww, bw):
    global _NC_CACHE
    if _NC_CACHE is None:
        _NC_CACHE = build_nc()
    nc = _NC_CACHE

    in_maps = make_in_maps(x, wq, bq, wk, bk, wv, ww)
    res = run_bass_kernel_spmd(nc, in_maps, core_ids=list(range(8)))

    const_row = (bv @ ww.T + bw).astype(np.float32)  # [768]
    out = np.empty((1, S, D), dtype=np.float32)
    for c in range(NC):
        acc_out = res.results[0 * NC + c]["out"] + res.results[1 * NC + c]["out"]
        out[0, c * SQ:(c + 1) * SQ, :] = acc_out + const_row
    return out
